# revision 12
# baseline (speedup 1.0000x reference)
"""Trainium2 Bass kernel for a custom LSTM.

Problem shapes (hardcoded): x [64, 1024, 512], 4 input-proj weights [512, 512],
4 hidden weights [512, 512], 4 biases [512]. Output [64, 1024, 512] fp32.

Strategy: data-parallel over batch across 8 NeuronCores (8 sequences per
core). Per core the kernel runs in 64 chunks of T=16 timesteps:

  phase-1 (per chunk): xg^T = Wx^T @ x_chunk^T  in bf16, gate-major layout
      [4H on partitions via 16 m-chunks, (t, b) on free], accumulated in
      PSUM and evacuated (+bias) to SBUF in fp32.
  scan (per step): g^T = Wh^T @ h^T via 64 small matmuls (Wh [128,128]
      bf16 chunks stationary, h^T [128, 8] moving), one PSUM bank per gate
      so VectorE can consume each gate's block while TensorE streams the
      next gate. Gates: tanh/sigmoid on ScalarE in [128, 32] tiles
      (H-major => full 128-partition utilization), state update on VectorE.
      h is written bf16 into a per-chunk history tile that doubles as the
      next step's matmul moving operand — no transposes inside the scan.
  flush (per chunk): 4 PE transposes turn the bf16 history back into
      batch-major [(t b), 512], upcast to fp32, single DMA to DRAM.

The sequential scan is the latency floor: ~64 ld-weights+matmul pairs per
step on the PE critical path; everything else overlaps under it.
"""

import sys

if "/opt/trn_rl_repo" not in sys.path:
    sys.path.insert(0, "/opt/trn_rl_repo")

import numpy as np
import ml_dtypes

import concourse.bass as bass
import concourse.bacc as bacc
import concourse.mybir as mybir
import concourse.tile as tile
from concourse.bass import ts, ds
from concourse.bass_utils import run_bass_kernel_spmd
from concourse.masks import make_identity

B, S, D, H = 64, 1024, 512, 512
NCORES = 8
BL = B // NCORES        # 8 sequences per core
G4 = 4 * H              # 2048 gate columns, order [g~, i, f, o]
T = 16                  # timesteps per chunk
F32 = mybir.dt.float32
BF16 = mybir.dt.bfloat16

AF = mybir.ActivationFunctionType
OP = mybir.AluOpType


def build_kernel(n_steps: int = S):
    assert n_steps % (2 * T) == 0
    nch = n_steps // T
    nc = bacc.Bacc(
        "TRN2", target_bir_lowering=False, debug=False, num_devices=NCORES
    )

    xp = nc.declare_dram_parameter("x", [BL, S + T, D], F32, isOutput=False)
    wx = nc.declare_dram_parameter("wx", [D, G4], BF16, isOutput=False)
    wh = nc.declare_dram_parameter("wh", [D, G4], BF16, isOutput=False)
    bias = nc.declare_dram_parameter("b", [G4], F32, isOutput=False)
    out = nc.declare_dram_parameter("out", [BL, n_steps, H], F32, isOutput=True)

    def x_chunk_ap(k):
        # rows ordered (t, b): row = t_local * BL + b
        return xp[:, ds(k * T, T), :].rearrange("b t d -> t b d")

    def out_chunk_ap(k):
        return out[:, ds(k * T, T), :].rearrange("b t d -> t b d")

    with tile.TileContext(nc) as tc:
        with (
            tc.tile_pool(name="const", bufs=1) as kpool,
            tc.tile_pool(name="state", bufs=1) as spool,
            tc.tile_pool(name="work", bufs=2) as wp,
            tc.tile_pool(name="ostage", bufs=2) as osp,
            tc.tile_pool(name="psg", bufs=1, space="PSUM") as psgp,
            tc.tile_pool(name="psxg", bufs=2, space="PSUM") as psxgp,
            tc.tile_pool(name="pstr", bufs=1, space="PSUM") as pstrp,
            tc.tile_pool(name="psot", bufs=1, space="PSUM") as psotp,
        ):
            # ---- constants ----
            wx_sb = kpool.tile([128, 4 * G4], BF16, name="wx_sb")
            wh_sb = kpool.tile([128, 4 * G4], BF16, name="wh_sb")
            bias_sb = kpool.tile([128, 16], F32, name="bias_sb")
            idf = kpool.tile([128, 128], F32, name="idf")
            idb = kpool.tile([128, 128], BF16, name="idb")
            for s in range(4):
                nc.sync.dma_start(out=wx_sb[:, ts(s, G4)], in_=wx[ts(s, 128), :])
                nc.sync.dma_start(out=wh_sb[:, ts(s, G4)], in_=wh[ts(s, 128), :])
            nc.sync.dma_start(
                out=bias_sb[:], in_=bias.rearrange("(m p) -> p m", p=128)
            )
            make_identity(nc, idf[:])
            make_identity(nc, idb[:])

            # ---- persistent state ----
            histA = spool.tile([128, 32 * T], BF16, name="histA")
            histB = spool.tile([128, 32 * T], BF16, name="histB")
            c_carry = spool.tile([128, 32], F32, name="c_carry")
            xtA = spool.tile([128, D], F32, name="xtA")
            xtB = spool.tile([128, D], F32, name="xtB")
            xgA = spool.tile([128, G4], F32, name="xgA")
            xgB = spool.tile([128, G4], F32, name="xgB")

            # hist layout: col = s*128 + t*8 + b  (s = H-slice, t = step-in-chunk)
            nc.gpsimd.memset(histB[:], 0.0)  # h_0 = 0
            nc.gpsimd.memset(c_carry[:], 0.0)               # c_0 = 0
            nc.sync.dma_start(out=xtB[:], in_=x_chunk_ap(0))

            def wtile(w_sb, s, m):
                return w_sb[:, s * G4 + m * 128 : s * G4 + (m + 1) * 128]

            def emit_chunk(kv, cur, prev, xt_rd, xt_wr, xg):
                # prefetch next chunk's x (input padded by T steps)
                nc.sync.dma_start(out=xt_wr[:], in_=x_chunk_ap(kv + 1))

                # ---- phase-1: xg^T for this chunk ----
                pstr = pstrp.tile([128, D], F32, name="pstr")
                for s in range(4):
                    nc.tensor.transpose(
                        pstr[:, ts(s, 128)], xt_rd[:, ts(s, 128)], idf[:]
                    )
                xTb = wp.tile([128, D], BF16, name="xTb")
                nc.vector.tensor_copy(xTb[:], pstr[:])
                for m in range(16):
                    psxg = psxgp.tile([128, 128], F32, name="psxg")
                    for s in range(4):
                        nc.tensor.matmul(
                            psxg[:],
                            lhsT=wtile(wx_sb, s, m),
                            rhs=xTb[:, ts(s, 128)],
                            start=(s == 0),
                            stop=(s == 3),
                        )
                    nc.vector.tensor_tensor(
                        xg[:, ts(m, 128)],
                        psxg[:],
                        bias_sb[:, m, None].to_broadcast((128, 128)),
                        OP.add,
                    )

                xg16 = xg[:].rearrange("p (m r) -> p m r", m=16)

                # ---- scan: T steps ----
                c_prev = c_carry
                for t in range(T):
                    if t == 0:
                        h_src, h_off = prev, (T - 1) * 8
                    else:
                        h_src, h_off = cur, (t - 1) * 8
                    # one PSUM bank (512 fp32) per gate: gate q at cols 512q
                    psg = psgp.tile([128, G4], F32, name="psg")
                    acts = {}
                    for q, (gname, func) in enumerate(
                        [
                            ("g", AF.Tanh),
                            ("i", AF.Sigmoid),
                            ("f", AF.Sigmoid),
                            ("o", AF.Sigmoid),
                        ]
                    ):
                        # each gate owns one PSUM bank; first matmul of the
                        # block clears the bank's has_written bits
                        for ml in range(4):
                            m = 4 * q + ml
                            for s in range(4):
                                nc.tensor.matmul(
                                    psg[:, 512 * q + 8 * ml : 512 * q + 8 * ml + 8],
                                    lhsT=wtile(wh_sb, s, m),
                                    rhs=h_src[:, s * 128 + h_off : s * 128 + h_off + 8],
                                    start=(ml == 0 and s == 0),
                                    stop=(ml == 3 and s == 3),
                                )
                        # gate block q finished: add xg slice, activate
                        pre = wp.tile([128, 32], F32, name=f"pre_{gname}")
                        nc.vector.tensor_tensor(
                            pre[:].rearrange("p (m b) -> p m b", m=4),
                            psg[:, 512 * q : 512 * q + 32].rearrange(
                                "p (m b) -> p m b", m=4
                            ),
                            xg16[:, 4 * q : 4 * q + 4, ts(t, 8)],
                            OP.add,
                        )
                        act = wp.tile([128, 32], F32, name=f"act_{gname}")
                        nc.scalar.activation(act[:], pre[:], func)
                        acts[gname] = act
                    t1 = wp.tile([128, 32], F32, name="t1")
                    nc.vector.tensor_tensor(t1[:], acts["i"][:], acts["g"][:], OP.mult)
                    t2 = wp.tile([128, 32], F32, name="t2")
                    nc.vector.tensor_tensor(t2[:], acts["f"][:], c_prev[:], OP.mult)
                    if t == T - 1:
                        c_new = c_carry
                    else:
                        c_new = wp.tile([128, 32], F32, name="c_new")
                    nc.vector.tensor_tensor(c_new[:], t1[:], t2[:], OP.add)
                    tch = wp.tile([128, 32], F32, name="tch")
                    nc.scalar.activation(tch[:], c_new[:], AF.Tanh)
                    h_out = cur[:].rearrange("p (s t b) -> p s t b", s=4, t=T, b=BL)[
                        :, :, t, :
                    ]
                    nc.vector.tensor_tensor(
                        h_out,
                        acts["o"][:].rearrange("p (s b) -> p s b", s=4),
                        tch[:].rearrange("p (s b) -> p s b", s=4),
                        OP.mult,
                    )
                    c_prev = c_new

                # ---- flush chunk outputs ----
                psot = psotp.tile([128, 512], BF16, name="psot")
                for s in range(4):
                    nc.tensor.transpose(psot[:, ts(s, 128)], cur[:, ts(s, 128)], idb[:])
                ost = osp.tile([128, 512], F32, name="ost")
                nc.vector.tensor_copy(ost[:], psot[:])
                nc.sync.dma_start(out=out_chunk_ap(kv), in_=ost[:])

            with tc.For_i(0, nch // 2, 1) as kk:
                emit_chunk(kk * 2, histA, histB, xtB, xtA, xgA)
                emit_chunk(kk * 2 + 1, histB, histA, xtA, xtB, xgB)

    nc.finalize()
    return nc


def _prep_inputs(x, W_ii, W_if, W_ig, W_io, W_hi, W_hf, W_hg, W_ho, b_i, b_f, b_g, b_o):
    # gate-block order [g~, i, f, o]
    Wx = np.ascontiguousarray(
        np.concatenate([W_ig, W_ii, W_if, W_io], axis=1)
    ).astype(ml_dtypes.bfloat16)
    Wh = np.ascontiguousarray(
        np.concatenate([W_hg, W_hi, W_hf, W_ho], axis=1)
    ).astype(ml_dtypes.bfloat16)
    bb = np.concatenate([b_g, b_i, b_f, b_o]).astype(np.float32)
    x = np.asarray(x, np.float32).reshape(NCORES, BL, S, D)
    xpad = np.zeros((NCORES, BL, S + T, D), np.float32)
    xpad[:, :, :S] = x
    return [
        {"x": np.ascontiguousarray(xpad[i]), "wx": Wx, "wh": Wh, "b": bb}
        for i in range(NCORES)
    ]


def run_spmd(inputs: dict, n_steps: int = S, trace: bool = False):
    in_maps = _prep_inputs(**inputs)
    nc = build_kernel(n_steps)
    res = run_bass_kernel_spmd(nc, in_maps, list(range(NCORES)), trace=trace)
    outp = np.concatenate(
        [np.asarray(res.results[i]["out"], np.float32) for i in range(NCORES)], axis=0
    )
    return outp, res


def kernel(**inputs) -> np.ndarray:
    outp, _ = run_spmd(inputs, S, trace=False)
    return outp


# revision 14
# speedup vs baseline: 1.3490x; 1.3490x over previous
"""Trainium2 Bass kernel for a custom LSTM.

Problem shapes (hardcoded): x [64, 1024, 512], 4 input-proj weights [512, 512],
4 hidden weights [512, 512], 4 biases [512]. Output [64, 1024, 512] fp32.

Strategy: data-parallel over batch across 8 NeuronCores (8 sequences per
core). Per core the kernel runs in 64 chunks of T=16 timesteps:

  phase-1 (per chunk): xg^T = Wx^T @ x_chunk^T  in bf16, gate-major layout
      [4H on partitions via 16 m-chunks, (t, b) on free], accumulated in
      PSUM and evacuated (+bias) to SBUF in fp32.
  scan (per step): g^T = Wh^T @ h^T via 64 small matmuls (Wh [128,128]
      bf16 chunks stationary, h^T [128, 8] moving), one PSUM bank per gate
      so VectorE can consume each gate's block while TensorE streams the
      next gate. Gates: tanh/sigmoid on ScalarE in [128, 32] tiles
      (H-major => full 128-partition utilization), state update on VectorE.
      h is written bf16 into a per-chunk history tile that doubles as the
      next step's matmul moving operand — no transposes inside the scan.
  flush (per chunk): 4 PE transposes turn the bf16 history back into
      batch-major [(t b), 512], upcast to fp32, single DMA to DRAM.

The sequential scan is the latency floor: ~64 ld-weights+matmul pairs per
step on the PE critical path; everything else overlaps under it.
"""

import sys

if "/opt/trn_rl_repo" not in sys.path:
    sys.path.insert(0, "/opt/trn_rl_repo")

import numpy as np
import ml_dtypes

import concourse.bass as bass
import concourse.bacc as bacc
import concourse.mybir as mybir
import concourse.tile as tile
from concourse.bass import ts, ds
from concourse.bass_utils import run_bass_kernel_spmd
from concourse.masks import make_identity

B, S, D, H = 64, 1024, 512, 512
NCORES = 8
BL = B // NCORES        # 8 sequences per core
G4 = 4 * H              # 2048 gate columns, order [g~, i, f, o]
T = 16                  # timesteps per chunk
F32 = mybir.dt.float32
BF16 = mybir.dt.bfloat16

AF = mybir.ActivationFunctionType
OP = mybir.AluOpType


def build_kernel(n_steps: int = S):
    assert n_steps % (2 * T) == 0
    nch = n_steps // T
    nc = bacc.Bacc(
        "TRN2", target_bir_lowering=False, debug=False, num_devices=NCORES
    )

    xp = nc.declare_dram_parameter("x", [BL, S + T, D], F32, isOutput=False)
    wx = nc.declare_dram_parameter("wx", [D, G4], BF16, isOutput=False)
    wh = nc.declare_dram_parameter("wh", [D, G4], BF16, isOutput=False)
    bias = nc.declare_dram_parameter("b", [G4], F32, isOutput=False)
    out = nc.declare_dram_parameter("out", [BL, n_steps, H], F32, isOutput=True)

    def x_chunk_ap(k):
        # rows ordered (t, b): row = t_local * BL + b
        return xp[:, ds(k * T, T), :].rearrange("b t d -> t b d")

    def out_chunk_ap(k):
        return out[:, ds(k * T, T), :].rearrange("b t d -> t b d")

    with tile.TileContext(nc) as tc:
        with (
            tc.tile_pool(name="const", bufs=1) as kpool,
            tc.tile_pool(name="state", bufs=1) as spool,
            tc.tile_pool(name="work", bufs=2) as wp,
            tc.tile_pool(name="ostage", bufs=2) as osp,
            tc.tile_pool(name="psg", bufs=1, space="PSUM") as psgp,
            tc.tile_pool(name="psxg", bufs=2, space="PSUM") as psxgp,
            tc.tile_pool(name="pstr", bufs=1, space="PSUM") as pstrp,
            tc.tile_pool(name="psot", bufs=1, space="PSUM") as psotp,
        ):
            # ---- constants ----
            wx_sb = kpool.tile([128, 4 * G4], BF16, name="wx_sb")
            wh_sb = kpool.tile([128, 4 * G4], BF16, name="wh_sb")
            bias_sb = kpool.tile([128, 16], F32, name="bias_sb")
            idf = kpool.tile([128, 128], F32, name="idf")
            idb = kpool.tile([128, 128], BF16, name="idb")
            for s in range(4):
                nc.sync.dma_start(out=wx_sb[:, ts(s, G4)], in_=wx[ts(s, 128), :])
                nc.sync.dma_start(out=wh_sb[:, ts(s, G4)], in_=wh[ts(s, 128), :])
            nc.sync.dma_start(
                out=bias_sb[:], in_=bias.rearrange("(m p) -> p m", p=128)
            )
            make_identity(nc, idf[:])
            make_identity(nc, idb[:])

            # ---- persistent state ----
            histA = spool.tile([128, 32 * T], BF16, name="histA")
            histB = spool.tile([128, 32 * T], BF16, name="histB")
            c_carry = spool.tile([128, 32], F32, name="c_carry")
            xtA = spool.tile([128, D], F32, name="xtA")
            xtB = spool.tile([128, D], F32, name="xtB")
            xgA = spool.tile([128, G4], BF16, name="xgA")
            xgB = spool.tile([128, G4], BF16, name="xgB")

            # hist layout: col = s*128 + t*8 + b  (s = H-slice, t = step-in-chunk)
            nc.gpsimd.memset(histB[:], 0.0)  # h_0 = 0
            nc.gpsimd.memset(c_carry[:], 0.0)               # c_0 = 0
            nc.sync.dma_start(out=xtB[:], in_=x_chunk_ap(0))

            def wtile(w_sb, s, m):
                return w_sb[:, s * G4 + m * 128 : s * G4 + (m + 1) * 128]

            def emit_chunk(kv, cur, prev, xt_rd, xt_wr, xg):
                # prefetch next chunk's x (input padded by T steps)
                nc.sync.dma_start(out=xt_wr[:], in_=x_chunk_ap(kv + 1))

                # ---- phase-1: xg^T for this chunk ----
                pstr = pstrp.tile([128, D], F32, name="pstr")
                for s in range(4):
                    nc.tensor.transpose(
                        pstr[:, ts(s, 128)], xt_rd[:, ts(s, 128)], idf[:]
                    )
                xTb = wp.tile([128, D], BF16, name="xTb")
                nc.vector.tensor_copy(xTb[:], pstr[:])
                for m in range(16):
                    psxg = psxgp.tile([128, 128], F32, name="psxg")
                    for s in range(4):
                        nc.tensor.matmul(
                            psxg[:],
                            lhsT=wtile(wx_sb, s, m),
                            rhs=xTb[:, ts(s, 128)],
                            start=(s == 0),
                            stop=(s == 3),
                        )
                    nc.vector.tensor_tensor(
                        xg[:, ts(m, 128)],
                        psxg[:],
                        bias_sb[:, m, None].to_broadcast((128, 128)),
                        OP.add,
                    )

                xg16 = xg[:].rearrange("p (m r) -> p m r", m=16)

                # ---- scan: T steps ----
                c_prev = c_carry
                for t in range(T):
                    if t == 0:
                        h_src, h_off = prev, (T - 1) * 8
                    else:
                        h_src, h_off = cur, (t - 1) * 8

                    # one separate PSUM tile (= one bank) per gate, so the
                    # gate blocks carry no false deps between each other
                    def gate_block(q, gname):
                        psq = psgp.tile([128, 32], F32, name=f"psg_{gname}")
                        # first write: inject xg via identity matmul
                        # (clears the bank's has_written bits, start=True)
                        nc.tensor.matmul(
                            psq[:],
                            lhsT=idb[:],
                            rhs=xg16[:, 4 * q : 4 * q + 4, ts(t, 8)],
                            start=True,
                            stop=False,
                        )
                        for ml in range(4):
                            m = 4 * q + ml
                            for s in range(4):
                                nc.tensor.matmul(
                                    psq[:, ts(ml, 8)],
                                    lhsT=wtile(wh_sb, s, m),
                                    rhs=h_src[:, s * 128 + h_off : s * 128 + h_off + 8],
                                    start=False,
                                    stop=(ml == 3 and s == 3),
                                )
                        return psq

                    def activate(psq, gname, func):
                        act = wp.tile([128, 32], F32, name=f"act_{gname}")
                        nc.scalar.activation(act[:], psq[:], func)
                        return act

                    ps_g = gate_block(0, "g")
                    tg = activate(ps_g, "g", AF.Tanh)
                    ps_i = gate_block(1, "i")
                    si = activate(ps_i, "i", AF.Sigmoid)
                    t1 = wp.tile([128, 32], F32, name="t1")
                    nc.vector.tensor_tensor(t1[:], si[:], tg[:], OP.mult)
                    ps_f = gate_block(2, "f")
                    sf = activate(ps_f, "f", AF.Sigmoid)
                    t2 = wp.tile([128, 32], F32, name="t2")
                    nc.vector.tensor_tensor(t2[:], sf[:], c_prev[:], OP.mult)
                    if t == T - 1:
                        c_new = c_carry
                    else:
                        c_new = wp.tile([128, 32], F32, name="c_new")
                    nc.vector.tensor_tensor(c_new[:], t1[:], t2[:], OP.add)
                    tch = wp.tile([128, 32], F32, name="tch")
                    nc.scalar.activation(tch[:], c_new[:], AF.Tanh)
                    ps_o = gate_block(3, "o")
                    acts = {"o": activate(ps_o, "o", AF.Sigmoid)}
                    h_out = cur[:].rearrange("p (s t b) -> p s t b", s=4, t=T, b=BL)[
                        :, :, t, :
                    ]
                    nc.vector.tensor_tensor(
                        h_out,
                        acts["o"][:].rearrange("p (s b) -> p s b", s=4),
                        tch[:].rearrange("p (s b) -> p s b", s=4),
                        OP.mult,
                    )
                    c_prev = c_new

                # ---- flush chunk outputs ----
                psot = psotp.tile([128, 512], BF16, name="psot")
                for s in range(4):
                    nc.tensor.transpose(psot[:, ts(s, 128)], cur[:, ts(s, 128)], idb[:])
                ost = osp.tile([128, 512], F32, name="ost")
                nc.vector.tensor_copy(ost[:], psot[:])
                nc.sync.dma_start(out=out_chunk_ap(kv), in_=ost[:])

            with tc.For_i(0, nch // 2, 1) as kk:
                emit_chunk(kk * 2, histA, histB, xtB, xtA, xgA)
                emit_chunk(kk * 2 + 1, histB, histA, xtA, xtB, xgB)

    nc.finalize()
    return nc


def _prep_inputs(x, W_ii, W_if, W_ig, W_io, W_hi, W_hf, W_hg, W_ho, b_i, b_f, b_g, b_o):
    # gate-block order [g~, i, f, o]
    Wx = np.ascontiguousarray(
        np.concatenate([W_ig, W_ii, W_if, W_io], axis=1)
    ).astype(ml_dtypes.bfloat16)
    Wh = np.ascontiguousarray(
        np.concatenate([W_hg, W_hi, W_hf, W_ho], axis=1)
    ).astype(ml_dtypes.bfloat16)
    bb = np.concatenate([b_g, b_i, b_f, b_o]).astype(np.float32)
    x = np.asarray(x, np.float32).reshape(NCORES, BL, S, D)
    xpad = np.zeros((NCORES, BL, S + T, D), np.float32)
    xpad[:, :, :S] = x
    return [
        {"x": np.ascontiguousarray(xpad[i]), "wx": Wx, "wh": Wh, "b": bb}
        for i in range(NCORES)
    ]


def run_spmd(inputs: dict, n_steps: int = S, trace: bool = False):
    in_maps = _prep_inputs(**inputs)
    nc = build_kernel(n_steps)
    res = run_bass_kernel_spmd(nc, in_maps, list(range(NCORES)), trace=trace)
    outp = np.concatenate(
        [np.asarray(res.results[i]["out"], np.float32) for i in range(NCORES)], axis=0
    )
    return outp, res


def kernel(**inputs) -> np.ndarray:
    outp, _ = run_spmd(inputs, S, trace=False)
    return outp


# revision 17
# speedup vs baseline: 1.4240x; 1.0556x over previous
"""Trainium2 Bass kernel for a custom LSTM.

Problem shapes (hardcoded): x [64, 1024, 512], 4 input-proj weights [512, 512],
4 hidden weights [512, 512], 4 biases [512]. Output [64, 1024, 512] fp32.

Strategy: data-parallel over batch across 8 NeuronCores (8 sequences per
core). Per core the kernel runs in 64 chunks of T=16 timesteps:

  phase-1 (per chunk): xg^T = Wx^T @ x_chunk^T  in bf16, gate-major layout
      [4H on partitions via 16 m-chunks, (t, b) on free], accumulated in
      PSUM and evacuated (+bias) to SBUF in fp32.
  scan (per step): g^T = Wh^T @ h^T via 64 small matmuls (Wh [128,128]
      bf16 chunks stationary, h^T [128, 8] moving), one PSUM bank per gate
      so VectorE can consume each gate's block while TensorE streams the
      next gate. Gates: tanh/sigmoid on ScalarE in [128, 32] tiles
      (H-major => full 128-partition utilization), state update on VectorE.
      h is written bf16 into a per-chunk history tile that doubles as the
      next step's matmul moving operand — no transposes inside the scan.
  flush (per chunk): 4 PE transposes turn the bf16 history back into
      batch-major [(t b), 512], upcast to fp32, single DMA to DRAM.

The sequential scan is the latency floor: ~64 ld-weights+matmul pairs per
step on the PE critical path; everything else overlaps under it.
"""

import sys

if "/opt/trn_rl_repo" not in sys.path:
    sys.path.insert(0, "/opt/trn_rl_repo")

import numpy as np
import ml_dtypes

import concourse.bass as bass
import concourse.bacc as bacc
import concourse.mybir as mybir
import concourse.tile as tile
from concourse.bass import ts, ds
from concourse.bass_utils import run_bass_kernel_spmd
from concourse.masks import make_identity

B, S, D, H = 64, 1024, 512, 512
NCORES = 8
BL = B // NCORES        # 8 sequences per core
G4 = 4 * H              # 2048 gate columns, order [g~, i, f, o]
T = 16                  # timesteps per chunk
F32 = mybir.dt.float32
BF16 = mybir.dt.bfloat16

AF = mybir.ActivationFunctionType
OP = mybir.AluOpType


def build_kernel(n_steps: int = S):
    assert n_steps % (4 * T) == 0
    nch = n_steps // T
    nc = bacc.Bacc(
        "TRN2", target_bir_lowering=False, debug=False, num_devices=NCORES
    )

    xp = nc.declare_dram_parameter("x", [BL, S + T, D], F32, isOutput=False)
    wx = nc.declare_dram_parameter("wx", [D, G4], BF16, isOutput=False)
    wh = nc.declare_dram_parameter("wh", [D, G4], BF16, isOutput=False)
    bias = nc.declare_dram_parameter("b", [G4], F32, isOutput=False)
    out = nc.declare_dram_parameter("out", [BL, n_steps, H], F32, isOutput=True)

    def x_chunk_ap(k):
        # rows ordered (t, b): row = t_local * BL + b
        return xp[:, ds(k * T, T), :].rearrange("b t d -> t b d")

    def out_chunk_ap(k):
        return out[:, ds(k * T, T), :].rearrange("b t d -> t b d")

    with tile.TileContext(nc) as tc:
        with (
            tc.tile_pool(name="const", bufs=1) as kpool,
            tc.tile_pool(name="state", bufs=1) as spool,
            tc.tile_pool(name="work", bufs=2) as wp,
            tc.tile_pool(name="ostage", bufs=2) as osp,
            tc.tile_pool(name="psg", bufs=1, space="PSUM") as psgp,
            tc.tile_pool(name="psxg", bufs=2, space="PSUM") as psxgp,
            tc.tile_pool(name="pstr", bufs=1, space="PSUM") as pstrp,
            tc.tile_pool(name="psot", bufs=1, space="PSUM") as psotp,
        ):
            # ---- constants ----
            wx_sb = kpool.tile([128, 4 * G4], BF16, name="wx_sb")
            wh_sb = kpool.tile([128, 4 * G4], BF16, name="wh_sb")
            bias_sb = kpool.tile([128, 16], F32, name="bias_sb")
            idf = kpool.tile([128, 128], F32, name="idf")
            idb = kpool.tile([128, 128], BF16, name="idb")
            for s in range(4):
                nc.sync.dma_start(out=wx_sb[:, ts(s, G4)], in_=wx[ts(s, 128), :])
                nc.sync.dma_start(out=wh_sb[:, ts(s, G4)], in_=wh[ts(s, 128), :])
            nc.sync.dma_start(
                out=bias_sb[:], in_=bias.rearrange("(m p) -> p m", p=128)
            )
            make_identity(nc, idf[:])
            make_identity(nc, idb[:])

            # ---- persistent state ----
            histA = spool.tile([128, 32 * T], BF16, name="histA")
            histB = spool.tile([128, 32 * T], BF16, name="histB")
            c_carry = spool.tile([128, 32], F32, name="c_carry")
            xtA = spool.tile([128, D], F32, name="xtA")
            xtB = spool.tile([128, D], F32, name="xtB")
            xgA = spool.tile([128, G4], BF16, name="xgA")
            xgB = spool.tile([128, G4], BF16, name="xgB")

            # hist layout: col = s*128 + t*8 + b  (s = H-slice, t = step-in-chunk)
            nc.gpsimd.memset(histB[:], 0.0)  # h_0 = 0
            nc.gpsimd.memset(c_carry[:], 0.0)               # c_0 = 0
            nc.sync.dma_start(out=xtB[:], in_=x_chunk_ap(0))

            def wtile(w_sb, s, m):
                return w_sb[:, s * G4 + m * 128 : s * G4 + (m + 1) * 128]

            def emit_chunk(kv, cur, prev, xt_rd, xt_wr, xg):
                # prefetch next chunk's x (input padded by T steps)
                nc.sync.dma_start(out=xt_wr[:], in_=x_chunk_ap(kv + 1))

                # ---- phase-1: xg^T for this chunk ----
                pstr = pstrp.tile([128, D], F32, name="pstr")
                for s in range(4):
                    nc.tensor.transpose(
                        pstr[:, ts(s, 128)], xt_rd[:, ts(s, 128)], idf[:]
                    )
                xTb = wp.tile([128, D], BF16, name="xTb")
                nc.vector.tensor_copy(xTb[:], pstr[:])
                for g in range(4):
                    # one full PSUM bank = 4 m-chunks per evacuation group
                    psxg = psxgp.tile([128, 512], F32, name="psxg")
                    for ml in range(4):
                        m = 4 * g + ml
                        for s in range(4):
                            nc.tensor.matmul(
                                psxg[:, ts(ml, 128)],
                                lhsT=wtile(wx_sb, s, m),
                                rhs=xTb[:, ts(s, 128)],
                                start=(s == 0),
                                stop=(s == 3),
                            )
                    nc.vector.tensor_tensor(
                        xg[:, ts(g, 512)].rearrange("p (m r) -> p m r", m=4),
                        psxg[:].rearrange("p (m r) -> p m r", m=4),
                        bias_sb[:, 4 * g : 4 * g + 4, None].to_broadcast(
                            (128, 4, 128)
                        ),
                        OP.add,
                    )

                xg16 = xg[:].rearrange("p (m r) -> p m r", m=16)

                # ---- scan: T steps ----
                c_prev = c_carry
                for t in range(T):
                    if t == 0:
                        h_src, h_off = prev, (T - 1) * 8
                    else:
                        h_src, h_off = cur, (t - 1) * 8

                    # one separate PSUM tile (= one bank) per gate, so the
                    # gate blocks carry no false deps between each other
                    def gate_block(q, gname):
                        psq = psgp.tile([128, 32], F32, name=f"psg_{gname}")
                        # first write: inject xg via identity matmul
                        # (clears the bank's has_written bits, start=True)
                        nc.tensor.matmul(
                            psq[:],
                            lhsT=idb[:],
                            rhs=xg16[:, 4 * q : 4 * q + 4, ts(t, 8)],
                            start=True,
                            stop=False,
                        )
                        for ml in range(4):
                            m = 4 * q + ml
                            for s in range(4):
                                nc.tensor.matmul(
                                    psq[:, ts(ml, 8)],
                                    lhsT=wtile(wh_sb, s, m),
                                    rhs=h_src[:, s * 128 + h_off : s * 128 + h_off + 8],
                                    start=False,
                                    stop=(ml == 3 and s == 3),
                                )
                        return psq

                    def activate(psq, gname, func):
                        act = wp.tile([128, 32], F32, name=f"act_{gname}")
                        nc.scalar.activation(act[:], psq[:], func)
                        return act

                    ps_g = gate_block(0, "g")
                    tg = activate(ps_g, "g", AF.Tanh)
                    ps_i = gate_block(1, "i")
                    si = activate(ps_i, "i", AF.Sigmoid)
                    t1 = wp.tile([128, 32], F32, name="t1")
                    nc.vector.tensor_tensor(t1[:], si[:], tg[:], OP.mult)
                    ps_f = gate_block(2, "f")
                    sf = activate(ps_f, "f", AF.Sigmoid)
                    t2 = wp.tile([128, 32], F32, name="t2")
                    nc.vector.tensor_tensor(t2[:], sf[:], c_prev[:], OP.mult)
                    if t == T - 1:
                        c_new = c_carry
                    else:
                        c_new = wp.tile([128, 32], F32, name="c_new")
                    nc.vector.tensor_tensor(c_new[:], t1[:], t2[:], OP.add)
                    tch = wp.tile([128, 32], F32, name="tch")
                    nc.scalar.activation(tch[:], c_new[:], AF.Tanh)
                    ps_o = gate_block(3, "o")
                    acts = {"o": activate(ps_o, "o", AF.Sigmoid)}
                    h_out = cur[:].rearrange("p (s t b) -> p s t b", s=4, t=T, b=BL)[
                        :, :, t, :
                    ]
                    nc.vector.tensor_tensor(
                        h_out,
                        acts["o"][:].rearrange("p (s b) -> p s b", s=4),
                        tch[:].rearrange("p (s b) -> p s b", s=4),
                        OP.mult,
                    )
                    c_prev = c_new

                # ---- flush chunk outputs ----
                psot = psotp.tile([128, 512], BF16, name="psot")
                for s in range(4):
                    nc.tensor.transpose(psot[:, ts(s, 128)], cur[:, ts(s, 128)], idb[:])
                ost = osp.tile([128, 512], F32, name="ost")
                nc.vector.tensor_copy(ost[:], psot[:])
                nc.sync.dma_start(out=out_chunk_ap(kv), in_=ost[:])

            # dummy sigmoid at body start: pins the ACT table set that holds
            # BOTH sigmoid and tanh, so the body needs only one table load
            dummy = spool.tile([128, 1], F32, name="dummy")

            with tc.For_i(0, nch // 4, 1) as kk:
                nc.scalar.activation(dummy[:], dummy[:], AF.Sigmoid)
                emit_chunk(kk * 4, histA, histB, xtB, xtA, xgA)
                emit_chunk(kk * 4 + 1, histB, histA, xtA, xtB, xgB)
                emit_chunk(kk * 4 + 2, histA, histB, xtB, xtA, xgA)
                emit_chunk(kk * 4 + 3, histB, histA, xtA, xtB, xgB)

    nc.finalize()
    return nc


def _prep_inputs(x, W_ii, W_if, W_ig, W_io, W_hi, W_hf, W_hg, W_ho, b_i, b_f, b_g, b_o):
    # gate-block order [g~, i, f, o]
    Wx = np.ascontiguousarray(
        np.concatenate([W_ig, W_ii, W_if, W_io], axis=1)
    ).astype(ml_dtypes.bfloat16)
    Wh = np.ascontiguousarray(
        np.concatenate([W_hg, W_hi, W_hf, W_ho], axis=1)
    ).astype(ml_dtypes.bfloat16)
    bb = np.concatenate([b_g, b_i, b_f, b_o]).astype(np.float32)
    x = np.asarray(x, np.float32).reshape(NCORES, BL, S, D)
    xpad = np.zeros((NCORES, BL, S + T, D), np.float32)
    xpad[:, :, :S] = x
    return [
        {"x": np.ascontiguousarray(xpad[i]), "wx": Wx, "wh": Wh, "b": bb}
        for i in range(NCORES)
    ]


def run_spmd(inputs: dict, n_steps: int = S, trace: bool = False):
    in_maps = _prep_inputs(**inputs)
    nc = build_kernel(n_steps)
    res = run_bass_kernel_spmd(nc, in_maps, list(range(NCORES)), trace=trace)
    outp = np.concatenate(
        [np.asarray(res.results[i]["out"], np.float32) for i in range(NCORES)], axis=0
    )
    return outp, res


def kernel(**inputs) -> np.ndarray:
    outp, _ = run_spmd(inputs, S, trace=False)
    return outp


# revision 18
# speedup vs baseline: 1.4390x; 1.0106x over previous
"""Trainium2 Bass kernel for a custom LSTM.

Problem shapes (hardcoded): x [64, 1024, 512], 4 input-proj weights [512, 512],
4 hidden weights [512, 512], 4 biases [512]. Output [64, 1024, 512] fp32.

Strategy: data-parallel over batch across 8 NeuronCores (8 sequences per
core). Per core the kernel runs in 64 chunks of T=16 timesteps:

  phase-1 (per chunk): xg^T = Wx^T @ x_chunk^T  in bf16, gate-major layout
      [4H on partitions via 16 m-chunks, (t, b) on free], accumulated in
      PSUM and evacuated (+bias) to SBUF in fp32.
  scan (per step): g^T = Wh^T @ h^T via 64 small matmuls (Wh [128,128]
      bf16 chunks stationary, h^T [128, 8] moving), one PSUM bank per gate
      so VectorE can consume each gate's block while TensorE streams the
      next gate. Gates: tanh/sigmoid on ScalarE in [128, 32] tiles
      (H-major => full 128-partition utilization), state update on VectorE.
      h is written bf16 into a per-chunk history tile that doubles as the
      next step's matmul moving operand — no transposes inside the scan.
  flush (per chunk): 4 PE transposes turn the bf16 history back into
      batch-major [(t b), 512], upcast to fp32, single DMA to DRAM.

The sequential scan is the latency floor: ~64 ld-weights+matmul pairs per
step on the PE critical path; everything else overlaps under it.
"""

import sys

if "/opt/trn_rl_repo" not in sys.path:
    sys.path.insert(0, "/opt/trn_rl_repo")

import numpy as np
import ml_dtypes

import concourse.bass as bass
import concourse.bacc as bacc
import concourse.mybir as mybir
import concourse.tile as tile
from concourse.bass import ts, ds
from concourse.bass_utils import run_bass_kernel_spmd
from concourse.masks import make_identity

B, S, D, H = 64, 1024, 512, 512
NCORES = 8
BL = B // NCORES        # 8 sequences per core
G4 = 4 * H              # 2048 gate columns, order [g~, i, f, o]
T = 16                  # timesteps per chunk
F32 = mybir.dt.float32
BF16 = mybir.dt.bfloat16

AF = mybir.ActivationFunctionType
OP = mybir.AluOpType


def build_kernel(n_steps: int = S):
    assert n_steps % (4 * T) == 0
    nch = n_steps // T
    nc = bacc.Bacc(
        "TRN2", target_bir_lowering=False, debug=False, num_devices=NCORES
    )

    xp = nc.declare_dram_parameter("x", [BL, S + T, D], F32, isOutput=False)
    wx = nc.declare_dram_parameter("wx", [D, G4], BF16, isOutput=False)
    wh = nc.declare_dram_parameter("wh", [D, G4], BF16, isOutput=False)
    bias = nc.declare_dram_parameter("b", [G4], F32, isOutput=False)
    out = nc.declare_dram_parameter("out", [BL, n_steps, H], F32, isOutput=True)

    def x_chunk_ap(k):
        # rows ordered (t, b): row = t_local * BL + b
        return xp[:, ds(k * T, T), :].rearrange("b t d -> t b d")

    def out_chunk_ap(k):
        return out[:, ds(k * T, T), :].rearrange("b t d -> t b d")

    with tile.TileContext(nc) as tc:
        with (
            tc.tile_pool(name="const", bufs=1) as kpool,
            tc.tile_pool(name="state", bufs=1) as spool,
            tc.tile_pool(name="work", bufs=2) as wp,
            tc.tile_pool(name="ostage", bufs=2) as osp,
            tc.tile_pool(name="psg", bufs=1, space="PSUM") as psgp,
            tc.tile_pool(name="psxg", bufs=2, space="PSUM") as psxgp,
            tc.tile_pool(name="pstr", bufs=1, space="PSUM") as pstrp,
            tc.tile_pool(name="psot", bufs=1, space="PSUM") as psotp,
        ):
            # ---- constants ----
            wx_sb = kpool.tile([128, 4 * G4], BF16, name="wx_sb")
            wh_sb = kpool.tile([128, 4 * G4], BF16, name="wh_sb")
            bias_sb = kpool.tile([128, 16], F32, name="bias_sb")
            idf = kpool.tile([128, 128], F32, name="idf")
            idb = kpool.tile([128, 128], BF16, name="idb")
            for s in range(4):
                nc.sync.dma_start(out=wx_sb[:, ts(s, G4)], in_=wx[ts(s, 128), :])
                nc.sync.dma_start(out=wh_sb[:, ts(s, G4)], in_=wh[ts(s, 128), :])
            nc.sync.dma_start(
                out=bias_sb[:], in_=bias.rearrange("(m p) -> p m", p=128)
            )
            make_identity(nc, idf[:])
            make_identity(nc, idb[:])

            # ---- persistent state ----
            histA = spool.tile([128, 32 * T], BF16, name="histA")
            histB = spool.tile([128, 32 * T], BF16, name="histB")
            c_carry = spool.tile([128, 32], F32, name="c_carry")
            xtA = spool.tile([128, D], F32, name="xtA")
            xtB = spool.tile([128, D], F32, name="xtB")
            xgA = spool.tile([128, G4], BF16, name="xgA")
            xgB = spool.tile([128, G4], BF16, name="xgB")

            # hist layout: col = s*128 + t*8 + b  (s = H-slice, t = step-in-chunk)
            nc.gpsimd.memset(histB[:], 0.0)  # h_0 = 0
            nc.gpsimd.memset(c_carry[:], 0.0)               # c_0 = 0
            nc.sync.dma_start(out=xtB[:], in_=x_chunk_ap(0))

            def wtile(w_sb, s, m):
                return w_sb[:, s * G4 + m * 128 : s * G4 + (m + 1) * 128]

            def emit_chunk(kv, cur, prev, xt_rd, xt_wr, xg):
                # prefetch next chunk's x (input padded by T steps)
                nc.sync.dma_start(out=xt_wr[:], in_=x_chunk_ap(kv + 1))

                # ---- phase-1: xg^T for this chunk ----
                pstr = pstrp.tile([128, D], F32, name="pstr")
                for s in range(4):
                    nc.tensor.transpose(
                        pstr[:, ts(s, 128)], xt_rd[:, ts(s, 128)], idf[:]
                    )
                xTb = wp.tile([128, D], BF16, name="xTb")
                nc.vector.tensor_copy(xTb[:], pstr[:])
                for g in range(4):
                    # one full PSUM bank = 4 m-chunks per evacuation group
                    psxg = psxgp.tile([128, 512], F32, name="psxg")
                    for ml in range(4):
                        m = 4 * g + ml
                        for s in range(4):
                            nc.tensor.matmul(
                                psxg[:, ts(ml, 128)],
                                lhsT=wtile(wx_sb, s, m),
                                rhs=xTb[:, ts(s, 128)],
                                start=(s == 0),
                                stop=(s == 3),
                            )
                    nc.vector.tensor_tensor(
                        xg[:, ts(g, 512)].rearrange("p (m r) -> p m r", m=4),
                        psxg[:].rearrange("p (m r) -> p m r", m=4),
                        bias_sb[:, 4 * g : 4 * g + 4, None].to_broadcast(
                            (128, 4, 128)
                        ),
                        OP.add,
                    )

                xg16 = xg[:].rearrange("p (m r) -> p m r", m=16)

                # ---- scan: T steps ----
                c_prev = c_carry
                for t in range(T):
                    if t == 0:
                        h_src, h_off = prev, (T - 1) * 8
                    else:
                        h_src, h_off = cur, (t - 1) * 8

                    # one separate PSUM tile (= one bank) per gate, so the
                    # gate blocks carry no false deps between each other
                    def gate_block(q, gname):
                        psq = psgp.tile([128, 32], F32, name=f"psg_{gname}")
                        # first write: inject xg via identity matmul
                        # (clears the bank's has_written bits, start=True)
                        nc.tensor.matmul(
                            psq[:],
                            lhsT=idb[:],
                            rhs=xg16[:, 4 * q : 4 * q + 4, ts(t, 8)],
                            start=True,
                            stop=False,
                        )
                        for ml in range(4):
                            m = 4 * q + ml
                            for s in range(4):
                                nc.tensor.matmul(
                                    psq[:, ts(ml, 8)],
                                    lhsT=wtile(wh_sb, s, m),
                                    rhs=h_src[:, s * 128 + h_off : s * 128 + h_off + 8],
                                    start=False,
                                    stop=(ml == 3 and s == 3),
                                )
                        return psq

                    def activate(psq, gname, func):
                        act = wp.tile([128, 32], F32, name=f"act_{gname}")
                        nc.scalar.activation(act[:], psq[:], func)
                        return act

                    ps_g = gate_block(0, "g")
                    tg = activate(ps_g, "g", AF.Tanh)
                    ps_i = gate_block(1, "i")
                    si = activate(ps_i, "i", AF.Sigmoid)
                    t1 = wp.tile([128, 32], F32, name="t1")
                    nc.vector.tensor_tensor(t1[:], si[:], tg[:], OP.mult)
                    ps_f = gate_block(2, "f")
                    sf = activate(ps_f, "f", AF.Sigmoid)
                    t2 = wp.tile([128, 32], F32, name="t2")
                    nc.vector.tensor_tensor(t2[:], sf[:], c_prev[:], OP.mult)
                    if t == T - 1:
                        c_new = c_carry
                    else:
                        c_new = wp.tile([128, 32], F32, name="c_new")
                    nc.vector.tensor_tensor(c_new[:], t1[:], t2[:], OP.add)
                    tch = wp.tile([128, 32], F32, name="tch")
                    nc.scalar.activation(tch[:], c_new[:], AF.Tanh)
                    ps_o = gate_block(3, "o")
                    acts = {"o": activate(ps_o, "o", AF.Sigmoid)}
                    h_out = cur[:].rearrange("p (s t b) -> p s t b", s=4, t=T, b=BL)[
                        :, :, t, :
                    ]
                    nc.vector.tensor_tensor(
                        h_out,
                        acts["o"][:].rearrange("p (s b) -> p s b", s=4),
                        tch[:].rearrange("p (s b) -> p s b", s=4),
                        OP.mult,
                    )
                    c_prev = c_new

                # ---- flush chunk outputs ----
                psot = psotp.tile([128, 512], BF16, name="psot")
                for s in range(4):
                    nc.tensor.transpose(psot[:, ts(s, 128)], cur[:, ts(s, 128)], idb[:])
                ost = osp.tile([128, 512], F32, name="ost")
                nc.vector.tensor_copy(ost[:], psot[:])
                nc.sync.dma_start(out=out_chunk_ap(kv), in_=ost[:])

            # dummy sigmoid at body start: pins the ACT table set that holds
            # BOTH sigmoid and tanh, so the body needs only one table load
            dummy = spool.tile([128, 1], F32, name="dummy")

            ET = mybir.EngineType
            with tc.For_i(
                0,
                nch // 4,
                1,
                hint_engines=(ET.PE, ET.Activation, ET.DVE, ET.SP, ET.Pool),
                staggered_reset=True,
            ) as kk:
                nc.scalar.activation(dummy[:], dummy[:], AF.Sigmoid)
                emit_chunk(kk * 4, histA, histB, xtB, xtA, xgA)
                emit_chunk(kk * 4 + 1, histB, histA, xtA, xtB, xgB)
                emit_chunk(kk * 4 + 2, histA, histB, xtB, xtA, xgA)
                emit_chunk(kk * 4 + 3, histB, histA, xtA, xtB, xgB)

    nc.finalize()
    return nc


def _prep_inputs(x, W_ii, W_if, W_ig, W_io, W_hi, W_hf, W_hg, W_ho, b_i, b_f, b_g, b_o):
    # gate-block order [g~, i, f, o]
    Wx = np.ascontiguousarray(
        np.concatenate([W_ig, W_ii, W_if, W_io], axis=1)
    ).astype(ml_dtypes.bfloat16)
    Wh = np.ascontiguousarray(
        np.concatenate([W_hg, W_hi, W_hf, W_ho], axis=1)
    ).astype(ml_dtypes.bfloat16)
    bb = np.concatenate([b_g, b_i, b_f, b_o]).astype(np.float32)
    x = np.asarray(x, np.float32).reshape(NCORES, BL, S, D)
    xpad = np.zeros((NCORES, BL, S + T, D), np.float32)
    xpad[:, :, :S] = x
    return [
        {"x": np.ascontiguousarray(xpad[i]), "wx": Wx, "wh": Wh, "b": bb}
        for i in range(NCORES)
    ]


def run_spmd(inputs: dict, n_steps: int = S, trace: bool = False):
    in_maps = _prep_inputs(**inputs)
    nc = build_kernel(n_steps)
    res = run_bass_kernel_spmd(nc, in_maps, list(range(NCORES)), trace=trace)
    outp = np.concatenate(
        [np.asarray(res.results[i]["out"], np.float32) for i in range(NCORES)], axis=0
    )
    return outp, res


def kernel(**inputs) -> np.ndarray:
    outp, _ = run_spmd(inputs, S, trace=False)
    return outp


# revision 19
# speedup vs baseline: 1.4485x; 1.0066x over previous
"""Trainium2 Bass kernel for a custom LSTM.

Problem shapes (hardcoded): x [64, 1024, 512], 4 input-proj weights [512, 512],
4 hidden weights [512, 512], 4 biases [512]. Output [64, 1024, 512] fp32.

Strategy: data-parallel over batch across 8 NeuronCores (8 sequences per
core). Per core the kernel runs in 64 chunks of T=16 timesteps:

  phase-1 (per chunk): xg^T = Wx^T @ x_chunk^T  in bf16, gate-major layout
      [4H on partitions via 16 m-chunks, (t, b) on free], accumulated in
      PSUM and evacuated (+bias) to SBUF in fp32.
  scan (per step): g^T = Wh^T @ h^T via 64 small matmuls (Wh [128,128]
      bf16 chunks stationary, h^T [128, 8] moving), one PSUM bank per gate
      so VectorE can consume each gate's block while TensorE streams the
      next gate. Gates: tanh/sigmoid on ScalarE in [128, 32] tiles
      (H-major => full 128-partition utilization), state update on VectorE.
      h is written bf16 into a per-chunk history tile that doubles as the
      next step's matmul moving operand — no transposes inside the scan.
  flush (per chunk): 4 PE transposes turn the bf16 history back into
      batch-major [(t b), 512], upcast to fp32, single DMA to DRAM.

The sequential scan is the latency floor: ~64 ld-weights+matmul pairs per
step on the PE critical path; everything else overlaps under it.
"""

import sys

if "/opt/trn_rl_repo" not in sys.path:
    sys.path.insert(0, "/opt/trn_rl_repo")

import numpy as np
import ml_dtypes

import concourse.bass as bass
import concourse.bacc as bacc
import concourse.mybir as mybir
import concourse.tile as tile
from concourse.bass import ts, ds
from concourse.bass_utils import run_bass_kernel_spmd
from concourse.masks import make_identity

B, S, D, H = 64, 1024, 512, 512
NCORES = 8
BL = B // NCORES        # 8 sequences per core
G4 = 4 * H              # 2048 gate columns, order [g~, i, f, o]
T = 16                  # timesteps per chunk
F32 = mybir.dt.float32
BF16 = mybir.dt.bfloat16

AF = mybir.ActivationFunctionType
OP = mybir.AluOpType


def build_kernel(n_steps: int = S):
    assert n_steps % (4 * T) == 0
    nch = n_steps // T
    nc = bacc.Bacc(
        "TRN2", target_bir_lowering=False, debug=False, num_devices=NCORES
    )

    xp = nc.declare_dram_parameter("x", [BL, S + T, D], F32, isOutput=False)
    wx = nc.declare_dram_parameter("wx", [D, G4], BF16, isOutput=False)
    wh = nc.declare_dram_parameter("wh", [D, G4], BF16, isOutput=False)
    bias = nc.declare_dram_parameter("b", [G4], F32, isOutput=False)
    out = nc.declare_dram_parameter("out", [BL, n_steps, H], F32, isOutput=True)

    def x_chunk_ap(k):
        # rows ordered (t, b): row = t_local * BL + b
        return xp[:, ds(k * T, T), :].rearrange("b t d -> t b d")

    def out_chunk_ap(k):
        return out[:, ds(k * T, T), :].rearrange("b t d -> t b d")

    with tile.TileContext(nc) as tc:
        with (
            tc.tile_pool(name="const", bufs=1) as kpool,
            tc.tile_pool(name="state", bufs=1) as spool,
            tc.tile_pool(name="work", bufs=2) as wp,
            tc.tile_pool(name="ostage", bufs=2) as osp,
            tc.tile_pool(name="psg", bufs=1, space="PSUM") as psgp,
            tc.tile_pool(name="psxg", bufs=2, space="PSUM") as psxgp,
            tc.tile_pool(name="pstr", bufs=1, space="PSUM") as pstrp,
            tc.tile_pool(name="psot", bufs=1, space="PSUM") as psotp,
        ):
            # ---- constants ----
            wx_sb = kpool.tile([128, 4 * G4], BF16, name="wx_sb")
            wh_sb = kpool.tile([128, 4 * G4], BF16, name="wh_sb")
            bias_sb = kpool.tile([128, 16], F32, name="bias_sb")
            idf = kpool.tile([128, 128], F32, name="idf")
            idb = kpool.tile([128, 128], BF16, name="idb")
            for s in range(4):
                nc.sync.dma_start(out=wx_sb[:, ts(s, G4)], in_=wx[ts(s, 128), :])
                nc.sync.dma_start(out=wh_sb[:, ts(s, G4)], in_=wh[ts(s, 128), :])
            nc.sync.dma_start(
                out=bias_sb[:], in_=bias.rearrange("(m p) -> p m", p=128)
            )
            make_identity(nc, idf[:])
            make_identity(nc, idb[:])

            # ---- persistent state ----
            histA = spool.tile([128, 32 * T], BF16, name="histA")
            histB = spool.tile([128, 32 * T], BF16, name="histB")
            c_carry = spool.tile([128, 32], F32, name="c_carry")
            xtA = spool.tile([128, D], F32, name="xtA")
            xtB = spool.tile([128, D], F32, name="xtB")
            xgA = spool.tile([128, G4], BF16, name="xgA")
            xgB = spool.tile([128, G4], BF16, name="xgB")

            # hist layout: col = s*128 + t*8 + b  (s = H-slice, t = step-in-chunk)
            nc.gpsimd.memset(histB[:], 0.0)  # h_0 = 0
            nc.gpsimd.memset(c_carry[:], 0.0)               # c_0 = 0
            nc.sync.dma_start(out=xtB[:], in_=x_chunk_ap(0))

            def wtile(w_sb, s, m):
                return w_sb[:, s * G4 + m * 128 : s * G4 + (m + 1) * 128]

            def emit_chunk(kv, cur, prev, xt_rd, xt_wr, xg):
                # prefetch next chunk's x (input padded by T steps)
                nc.sync.dma_start(out=xt_wr[:], in_=x_chunk_ap(kv + 1))

                # ---- phase-1: xg^T for this chunk ----
                pstr = pstrp.tile([128, D], F32, name="pstr")
                for s in range(4):
                    nc.tensor.transpose(
                        pstr[:, ts(s, 128)], xt_rd[:, ts(s, 128)], idf[:]
                    )
                xTb = wp.tile([128, D], BF16, name="xTb")
                nc.vector.tensor_copy(xTb[:], pstr[:])
                for g in range(4):
                    # one full PSUM bank = 4 m-chunks per evacuation group
                    psxg = psxgp.tile([128, 512], F32, name="psxg")
                    for ml in range(4):
                        m = 4 * g + ml
                        for s in range(4):
                            nc.tensor.matmul(
                                psxg[:, ts(ml, 128)],
                                lhsT=wtile(wx_sb, s, m),
                                rhs=xTb[:, ts(s, 128)],
                                start=(s == 0),
                                stop=(s == 3),
                            )
                    nc.vector.tensor_tensor(
                        xg[:, ts(g, 512)].rearrange("p (m r) -> p m r", m=4),
                        psxg[:].rearrange("p (m r) -> p m r", m=4),
                        bias_sb[:, 4 * g : 4 * g + 4, None].to_broadcast(
                            (128, 4, 128)
                        ),
                        OP.add,
                    )

                xg16 = xg[:].rearrange("p (m r) -> p m r", m=16)

                # ---- scan: T steps ----
                c_prev = c_carry
                for t in range(T):
                    if t == 0:
                        h_src, h_off = prev, (T - 1) * 8
                    else:
                        h_src, h_off = cur, (t - 1) * 8

                    # one separate PSUM tile (= one bank) per gate, so the
                    # gate blocks carry no false deps between each other
                    def gate_block(q, gname):
                        psq = psgp.tile([128, 32], F32, name=f"psg_{gname}")
                        # first write: inject xg via identity matmul
                        # (clears the bank's has_written bits, start=True)
                        nc.tensor.matmul(
                            psq[:],
                            lhsT=idb[:],
                            rhs=xg16[:, 4 * q : 4 * q + 4, ts(t, 8)],
                            start=True,
                            stop=False,
                        )
                        for ml in range(4):
                            m = 4 * q + ml
                            for s in range(4):
                                nc.tensor.matmul(
                                    psq[:, ts(ml, 8)],
                                    lhsT=wtile(wh_sb, s, m),
                                    rhs=h_src[:, s * 128 + h_off : s * 128 + h_off + 8],
                                    start=False,
                                    stop=(ml == 3 and s == 3),
                                )
                        return psq

                    def activate(psq, gname, func):
                        act = wp.tile([128, 32], F32, name=f"act_{gname}")
                        nc.scalar.activation(act[:], psq[:], func)
                        return act

                    ps_g = gate_block(0, "g")
                    tg = activate(ps_g, "g", AF.Tanh)
                    ps_i = gate_block(1, "i")
                    si = activate(ps_i, "i", AF.Sigmoid)
                    t1 = wp.tile([128, 32], F32, name="t1")
                    nc.vector.tensor_tensor(t1[:], si[:], tg[:], OP.mult)
                    ps_f = gate_block(2, "f")
                    sf = activate(ps_f, "f", AF.Sigmoid)
                    t2 = wp.tile([128, 32], F32, name="t2")
                    nc.vector.tensor_tensor(t2[:], sf[:], c_prev[:], OP.mult)
                    if t == T - 1:
                        c_new = c_carry
                    else:
                        c_new = wp.tile([128, 32], F32, name="c_new")
                    nc.vector.tensor_tensor(c_new[:], t1[:], t2[:], OP.add)
                    tch = wp.tile([128, 32], F32, name="tch")
                    nc.scalar.activation(tch[:], c_new[:], AF.Tanh)
                    ps_o = gate_block(3, "o")
                    acts = {"o": activate(ps_o, "o", AF.Sigmoid)}
                    h_out = cur[:].rearrange("p (s t b) -> p s t b", s=4, t=T, b=BL)[
                        :, :, t, :
                    ]
                    nc.vector.tensor_tensor(
                        h_out,
                        acts["o"][:].rearrange("p (s b) -> p s b", s=4),
                        tch[:].rearrange("p (s b) -> p s b", s=4),
                        OP.mult,
                    )
                    c_prev = c_new

                # ---- flush chunk outputs ----
                psot = psotp.tile([128, 512], BF16, name="psot")
                for s in range(4):
                    nc.tensor.transpose(psot[:, ts(s, 128)], cur[:, ts(s, 128)], idb[:])
                ost = osp.tile([128, 512], F32, name="ost")
                nc.vector.tensor_copy(ost[:], psot[:])
                nc.sync.dma_start(out=out_chunk_ap(kv), in_=ost[:])

            # dummy sigmoid at body start: pins the ACT table set that holds
            # BOTH sigmoid and tanh, so the body needs only one table load
            dummy = spool.tile([128, 1], F32, name="dummy")

            ET = mybir.EngineType
            body = 8 if nch % 8 == 0 else 4
            with tc.For_i(
                0,
                nch // body,
                1,
                hint_engines=(ET.PE, ET.Activation, ET.DVE, ET.SP, ET.Pool),
                staggered_reset=True,
            ) as kk:
                nc.scalar.activation(dummy[:], dummy[:], AF.Sigmoid)
                for j in range(body):
                    if j % 2 == 0:
                        emit_chunk(kk * body + j, histA, histB, xtB, xtA, xgA)
                    else:
                        emit_chunk(kk * body + j, histB, histA, xtA, xtB, xgB)

    nc.finalize()
    return nc


def _prep_inputs(x, W_ii, W_if, W_ig, W_io, W_hi, W_hf, W_hg, W_ho, b_i, b_f, b_g, b_o):
    # gate-block order [g~, i, f, o]
    Wx = np.ascontiguousarray(
        np.concatenate([W_ig, W_ii, W_if, W_io], axis=1)
    ).astype(ml_dtypes.bfloat16)
    Wh = np.ascontiguousarray(
        np.concatenate([W_hg, W_hi, W_hf, W_ho], axis=1)
    ).astype(ml_dtypes.bfloat16)
    bb = np.concatenate([b_g, b_i, b_f, b_o]).astype(np.float32)
    x = np.asarray(x, np.float32).reshape(NCORES, BL, S, D)
    xpad = np.zeros((NCORES, BL, S + T, D), np.float32)
    xpad[:, :, :S] = x
    return [
        {"x": np.ascontiguousarray(xpad[i]), "wx": Wx, "wh": Wh, "b": bb}
        for i in range(NCORES)
    ]


def run_spmd(inputs: dict, n_steps: int = S, trace: bool = False):
    in_maps = _prep_inputs(**inputs)
    nc = build_kernel(n_steps)
    res = run_bass_kernel_spmd(nc, in_maps, list(range(NCORES)), trace=trace)
    outp = np.concatenate(
        [np.asarray(res.results[i]["out"], np.float32) for i in range(NCORES)], axis=0
    )
    return outp, res


def kernel(**inputs) -> np.ndarray:
    outp, _ = run_spmd(inputs, S, trace=False)
    return outp
